# revision 31
# baseline (speedup 1.0000x reference)
"""Multi-head self-attention kernel for Trainium2, length-balanced over 8 NeuronCores.

Problem: B=8, S=1024, IN_DIM=D_MODEL=768, H=12, DK=64.
  q/k/v = Q @ W{q,k,v}.T + b   -> [b, H, s, dk]
  scores = exp(q k^T / 8) * key_mask ; attn = scores / (sum + 1e-8)
  out = attn @ v -> [b, s, 768]

Sharding: scores/exp/ctx work for batch b scales with w(b)=ceil(length[b]/128)
key tiles. Batches are sorted by w and paired heavy+light (rank g with rank
7-g); each pair of batches is served by two cores, each taking 3 of the 6
head-pair d-tiles of BOTH batches. Every core runs the same SPMD program with
slot bounds [sH,sH,sH,sL,sL,sL] (sH = max heavy w, sL = max light w), so the
per-core key-tile work drops from 6*max(w) to 3*(sH+sL).

Per slot (batch b, local d-tile t):
  - qT/kT [128,1024] via 24 matmuls N=512 (lhsT = weight k-tiles)
  - scoresT[sk,sq] psum via K=64 matmuls (two heads packed in PE rows
    0-63 / 64-127), sk < bound only
  - exp fused with per-partition mask bias + 1/sqrt(dk) scale on ACT -> bf16
  - ctx psum[sq,65] accumulated over sk; col 64 = rowsum (ones col in v);
    DVE normalizes into out_sb
  - v in [s, d] layout, 65-stride heads with ones col, rows only up to bound
  - pipeline: ctx of slot j-1 (and vproj) ride under scores of slot j
Host gathers the per-core [2, 8sq, 128, 384] outputs into [8, 1024, 768].
"""

import functools
import sys
import types

import numpy as np

B, S, IN_DIM, D_MODEL, H = 8, 1024, 768, 768, 12
DK = D_MODEL // H
NCORES = 8
NKT = IN_DIM // 128   # 6 contraction tiles
NDT = D_MODEL // 128  # 6 d-tiles (head pairs)
NST = S // 128        # 8 s-tiles
NPAIR = 3             # head-pairs per core (half of NDT)
VW = NPAIR * 2 * (DK + 1)   # 390: v columns per core-batch (6 heads x 65)
OW = NPAIR * 128            # 384: out columns per core-batch
MASK_BIAS = -60.0


def _install_shims():
    """antenv.axon_hooks shim (for NTFF tracing) + Tile drain-wait splitting
    (this walrus build accepts only one sync-wait command per Drain/CTRL)."""
    if 'antenv.axon_hooks' not in sys.modules:
        mod = types.ModuleType('antenv.axon_hooks')
        mod._hook = None
        mod.set_axon_ntff_profile_hook = lambda h: setattr(mod, '_hook', h)
        mod.get_axon_ntff_profile_hook = lambda: mod._hook
        sys.modules['antenv.axon_hooks'] = mod
        try:
            import antenv
            antenv.axon_hooks = mod
            from trn_agent_boot.trn_boot import _ntff_profile_via_ctypes
            mod.set_axon_ntff_profile_hook(
                _ntff_profile_via_ctypes('/opt/axon/libaxon_pjrt.so'))
        except Exception:
            pass

    import concourse.tile as tile
    if getattr(tile.TileContext, '_drain_patched', False):
        return
    from concourse.vector_clock import ScopedClock, VectorClock

    def _patched_drain_and_barrier(self, tick_clock, wait_clock):
        nc = self.nc
        gvec = tick_clock.global_clock
        n = len(gvec)
        # Only the DMA/collective proc lanes (>=10) complete asynchronously;
        # engine lanes are fenced by the barrier's own arrival. Spread the
        # one-wait-per-NOP chains across queues so they drain in parallel.
        engs = [nc.sync, nc.gpsimd, nc.scalar, nc.vector, nc.tensor]
        ei = 0
        for i in range(n):
            t = gvec[i]
            if t <= 0 or i < 10:
                continue
            v = [0] * n
            v[i] = t
            nop = engs[ei % len(engs)].nop(nofuse=True, hint="drain_wait_split")
            ei += 1
            wait_clock.add_sem_waits(nop.ins, ScopedClock({None: VectorClock(v)}))
        nc.sync.drain()
        nc.all_engine_barrier()
        assert self.sems is not None
        popped = nc._tile_sem_poison_stack.pop()
        assert popped is self._sem_poison
        # Program ends here: skip the semaphore clears + second barrier
        # (pure epilogue hygiene for nested scopes; outputs are already
        # fenced by the drain + barrier above). Keep host-side bookkeeping.
        sems = list(self.sems.allocated().values())
        nc._state.prepend_free_semaphores(
            [s.num if hasattr(s, 'num') else s for s in sems])

    tile.TileContext._drain_and_barrier = _patched_drain_and_barrier

    # This walrus build accepts at most ONE sync-wait command per engine
    # instruction: split extra waits onto non-fusable NOPs emitted just
    # before the instruction on the same engine queue.
    import bass_rust
    import concourse.mybir as mybir
    _orig_lower = tile.TileContext._lower_ordered_insts

    def _split_waits_then_lower(self, ordered):
        nc = self.nc
        for bbname, insts in ordered.items():
            need = any(
                i.sync_info is not None and i.sync_info.on_wait
                and len(i.sync_info.on_wait) > 1
                for i in insts)
            if not need:
                continue
            out = []
            for inst in insts:
                si = inst.sync_info
                if si is not None and si.on_wait and len(si.on_wait) > 1:
                    waits = list(si.on_wait)
                    for w in waits[:-1]:
                        nop = mybir.InstNoOp(
                            name=nc.get_next_instruction_name(), ins=[], outs=[])
                        nop.engine = inst.engine
                        nop.bass_nofuse = True
                        nop.sync_info = bass_rust.SyncInfo(
                            on_wait=[w], on_update=[])
                        out.append(nop)
                    inst.sync_info = bass_rust.SyncInfo(
                        on_wait=[waits[-1]],
                        on_update=list(si.on_update or []))
                out.append(inst)
            insts[:] = out
        return _orig_lower(self, ordered)

    tile.TileContext._lower_ordered_insts = _split_waits_then_lower
    tile.TileContext._drain_patched = True


@functools.lru_cache(maxsize=None)
def _build_program(s_hi: int, s_lo: int, use_bias: bool):
    import concourse.bass as bass
    import concourse.tile as tile
    import concourse.mybir as mybir
    from contextlib import ExitStack

    f32 = mybir.dt.float32
    bf16 = mybir.dt.bfloat16
    EXP = mybir.ActivationFunctionType.Exp

    # slot -> (batch index, sk bound); light batch last keeps the tail short
    slots = [(0, s_hi)] * NPAIR + [(1, s_lo)] * NPAIR
    bounds = {0: s_hi, 1: s_lo}

    nc = bass.Bass("TRN2", enable_partition_id=False)
    qt_d = nc.dram_tensor("qt", (2, IN_DIM, S), bf16, kind="ExternalInput")
    wqm_d = nc.dram_tensor("wqm", (NPAIR, 128, NKT, 128), bf16, kind="ExternalInput")
    wkm_d = nc.dram_tensor("wkm", (NPAIR, 128, NKT, 128), bf16, kind="ExternalInput")
    wvt_d = nc.dram_tensor("wvt", (NKT, 128, OW), bf16, kind="ExternalInput")
    mb_d = nc.dram_tensor("mb", (128, 2, NST), f32, kind="ExternalInput")
    if use_bias:
        bq_d = nc.dram_tensor("bq", (1, OW), bf16, kind="ExternalInput")
        bk_d = nc.dram_tensor("bk", (1, OW), bf16, kind="ExternalInput")
        bv_d = nc.dram_tensor("bv", (1, OW), bf16, kind="ExternalInput")
    # partition-major output: [batch][partition, sq*OW + col] so the DMA
    # moves >=3KB contiguous per partition row; host un-permutes
    out_d = nc.dram_tensor("out", (2, 128, NST * OW), f32, kind="ExternalOutput")

    with tile.TileContext(nc) as tc, ExitStack() as ctx:
        const = ctx.enter_context(tc.tile_pool(name="const", bufs=1))
        big = ctx.enter_context(tc.tile_pool(name="big", bufs=1))
        qkpool = ctx.enter_context(tc.tile_pool(name="qk", bufs=3))
        smpool = ctx.enter_context(tc.tile_pool(name="sm", bufs=4))
        pj = ctx.enter_context(tc.tile_pool(name="pj", bufs=4, space="PSUM"))
        sc = ctx.enter_context(tc.tile_pool(name="sc", bufs=2, space="PSUM"))

        # ---- input DMA: critical path first (mb, wq0/wk0, qtA), the rest
        # (wvt, remaining weights) behind; qtB is deferred into slot 0's
        # pipeline so it doesn't steal HBM bandwidth from qtA
        mb_sb = const.tile([128, 2, NST], f32)
        nc.gpsimd.dma_start(out=mb_sb, in_=mb_d[:, :, :])

        wq_big = big.tile([128, NPAIR, NKT, 128], bf16, name="wq")
        wk_big = big.tile([128, NPAIR, NKT, 128], bf16, name="wk")
        wq_sb = [wq_big[:, t] for t in range(NPAIR)]
        wk_sb = [wk_big[:, t] for t in range(NPAIR)]
        nc.scalar.dma_start(out=wq_big[:, 0], in_=wqm_d[0])
        nc.scalar.dma_start(out=wk_big[:, 0], in_=wkm_d[0])

        qt_big = [big.tile([128, NKT, S], bf16, name=f"qt{b}") for b in range(2)]
        qt_sb = [[qt_big[b][:, k] for k in range(NKT)] for b in range(2)]
        for k in range(NKT):
            eng = nc.sync if k % 2 == 0 else nc.gpsimd
            eng.dma_start(out=qt_big[0][:, k], in_=qt_d[0, k * 128:(k + 1) * 128, :])
        wvt_big = big.tile([128, NKT, OW], bf16, name="wvt")
        wvt_sb = [wvt_big[:, k] for k in range(NKT)]
        for k in range(NKT):
            nc.gpsimd.dma_start(out=wvt_big[:, k], in_=wvt_d[k])
        for t in range(1, NPAIR):
            nc.scalar.dma_start(out=wq_big[:, t], in_=wqm_d[t])
            nc.scalar.dma_start(out=wk_big[:, t], in_=wkm_d[t])

        def emit_qtb_dma(k):
            eng = nc.sync if k % 2 == 0 else nc.gpsimd
            eng.dma_start(out=qt_big[1][:, k], in_=qt_d[1, k * 128:(k + 1) * 128, :])

        if use_bias:
            ones_sb = const.tile([1, 512], bf16)
            nc.vector.memset(ones_sb, 1.0)
            bq_sb = const.tile([1, OW], bf16)
            nc.sync.dma_start(out=bq_sb, in_=bq_d[:, :])
            bk_sb = const.tile([1, OW], bf16)
            nc.sync.dma_start(out=bk_sb, in_=bk_d[:, :])
            bv_sb = const.tile([1, OW], bf16)
            nc.sync.dma_start(out=bv_sb, in_=bv_d[:, :])

        v_sb = [big.tile([128, bounds[b], VW], bf16, name=f"vsb{b}")
                for b in range(2)]
        out_sb = [big.tile([128, NST, OW], f32, name=f"out{b}")
                  for b in range(2)]
        probs_big = [big.tile([128, 2, NST, S], bf16, name=f"probs{par}")
                     for par in range(2)]

        # ---- v projection for (batch b, s-row tile): [s, d] layout,
        # heads strided by DK+1 with a ones column appended
        def emit_vproj(b, srow):
            ps = pj.tile([128, NPAIR * 2 * DK], f32, tag="px",
                         name=f"psv{b}_{srow}")
            for k in range(NKT):
                nc.tensor.matmul(
                    ps,
                    lhsT=qt_sb[b][k][:, srow * 128:(srow + 1) * 128],
                    rhs=wvt_sb[k],
                    start=(k == 0), stop=(k == NKT - 1 and not use_bias))
            if use_bias:
                nc.tensor.matmul(
                    ps, lhsT=ones_sb[0:1, 0:128], rhs=bv_sb,
                    start=False, stop=True)
            dst = v_sb[b][:, srow, :]
            dst3 = dst.rearrange("p (h x) -> p h x", x=DK + 1)[:, :, 0:DK]
            src3 = ps.rearrange("p (h x) -> p h x", x=DK)
            nc.vector.tensor_copy(out=dst3, in_=src3)
            ones_dst = dst.rearrange("p (h x) -> p h x", x=DK + 1)[:, :, DK:DK + 1]
            nc.vector.memset(ones_dst, 1.0)

        # ---- qT/kT projection for slot (batch b, local pair t)
        def emit_qkproj(b, t):
            qT = qkpool.tile([128, S], bf16, tag="qT", name=f"qT{b}_{t}")
            kT = qkpool.tile([128, S], bf16, tag="kT", name=f"kT{b}_{t}")
            # kT is only consumed up to bound*128 key columns by scores
            nch_k = -(-(bounds[b] * 128) // 512)
            for w_sb, bias_nm, dstT, nm in (
                    (wq_sb[t], "bq", qT, "q"), (wk_sb[t], "bk", kT, "k")):
                for nch in range(2 if nm == "q" else nch_k):
                    ps = pj.tile([128, 512], f32, tag="px",
                                 name=f"ps{nm}{b}_{t}_{nch}")
                    for k in range(NKT):
                        nc.tensor.matmul(
                            ps,
                            lhsT=w_sb[:, k, :],
                            rhs=qt_sb[b][k][:, nch * 512:(nch + 1) * 512],
                            start=(k == 0), stop=(k == NKT - 1 and not use_bias))
                    if use_bias:
                        bias_sb = bq_sb if bias_nm == "bq" else bk_sb
                        nc.tensor.matmul(
                            ps,
                            lhsT=bias_sb[0:1, t * 128:(t + 1) * 128],
                            rhs=ones_sb[0:1, 0:512],
                            start=False, stop=True)
                    nc.vector.tensor_copy(
                        out=dstT[:, nch * 512:(nch + 1) * 512], in_=ps)
            return qT, kT

        probs = {}

        def emit_scores_sk(j, b, sk, qT, kT):
            pss = []
            for hl in range(2):
                pss.append(sc.tile([128, S], f32, tag="sc",
                                   name=f"sc{j}_{sk}_{hl}"))
            # nch outer / hl inner: adjacent matmuls target different PE
            # row-groups (partitions 0-63 vs 64-127) and run concurrently
            for nch in range(2):
                for hl in range(2):
                    lo, hi = hl * 64, (hl + 1) * 64
                    nc.tensor.matmul(
                        pss[hl][:, nch * 512:(nch + 1) * 512],
                        lhsT=kT[lo:hi, sk * 128:(sk + 1) * 128],
                        rhs=qT[lo:hi, nch * 512:(nch + 1) * 512],
                        start=True, stop=True)
            for hl in range(2):
                pb = probs_big[j % 2][:, hl, sk]
                nc.scalar.activation(
                    out=pb, in_=pss[hl], func=EXP,
                    bias=mb_sb[:, b, sk:sk + 1], scale=1.0 / np.sqrt(DK))
                probs[(j % 2, hl, sk)] = pb

        def emit_ctx_group(j, b, t, g):
            sq, hl = g // 2, g % 2
            hloc = 2 * t + hl
            n_sk = bounds[b]
            pc = pj.tile([128, DK + 1], f32, tag="px", name=f"cx{j}_{g}")
            for sk in range(n_sk):
                nc.tensor.matmul(
                    pc,
                    lhsT=probs[(j % 2, hl, sk)][:, sq * 128:(sq + 1) * 128],
                    rhs=v_sb[b][:, sk, hloc * (DK + 1):(hloc + 1) * (DK + 1)],
                    start=(sk == 0), stop=(sk == n_sk - 1))
            rc = smpool.tile([128, 1], f32, tag="rc", name=f"rc{j}_{g}")
            nc.vector.tensor_scalar_add(rc, pc[:, DK:DK + 1], 1e-8)
            nc.vector.reciprocal(rc, rc)
            dst = out_sb[b][:, sq, hloc * DK:(hloc + 1) * DK]
            if b == 1:
                # light-batch ctx is DVE-normalize paced; ACT is idle there
                nc.scalar.mul(dst, pc[:, 0:DK], rc)
            else:
                nc.vector.tensor_scalar_mul(dst, pc[:, 0:DK], rc)

        def emit_out_dma(b, sqh):
            # one DMA per half batch: [128, 4*OW] = 6KB per partition row
            # (bigger descriptors; DMA engines are descriptor-rate bound)
            eng = (nc.gpsimd, nc.sync)[(b * 2 + sqh) % 2]
            eng.dma_start(
                out=out_d[b][:, sqh * 4 * OW:(sqh + 1) * 4 * OW],
                in_=out_sb[b][:, sqh * 4:(sqh + 1) * 4, :])

        # ---- main pipeline: slot j runs qkproj + scores; ctx of slot j-1
        # (and vproj of a freshly started batch) ride along its sk loop
        def carried_work(j):
            """List of closures to interleave under slot j's sk loop."""
            work = []
            if j == 0:
                work += [functools.partial(emit_vproj, 0, sr)
                         for sr in range(s_hi)]
                work += [functools.partial(emit_qtb_dma, k)
                         for k in range(NKT)]
            if j == 2:
                work += [functools.partial(emit_vproj, 1, sr)
                         for sr in range(s_lo)]
            if j > 0:
                pb, pt = slots[j - 1][0], (j - 1) % NPAIR
                for g in range(2 * NST):
                    work.append(functools.partial(emit_ctx_group, j - 1, pb, pt, g))
                    if pt == NPAIR - 1 and g % 8 == 7:
                        # batch pb's out columns complete for this half
                        work.append(functools.partial(emit_out_dma, pb, g // 8))
            return work

        for j, (b, bound) in enumerate(slots):
            qT, kT = emit_qkproj(b, j % NPAIR)
            work = carried_work(j)
            wi = 0
            for sk in range(bound):
                emit_scores_sk(j, b, sk, qT, kT)
                target = len(work) * (sk + 1) // bound
                while wi < target:
                    work[wi]()
                    wi += 1
            while wi < len(work):
                work[wi]()
                wi += 1

        # tail: ctx + output of the last (light) slot
        j = len(slots) - 1
        for g in range(2 * NST):
            emit_ctx_group(j, 1, NPAIR - 1, g)
            if g % 8 == 7:
                emit_out_dma(1, g // 8)

    return nc


TRACE = False
LAST_EXEC_NS = None
LAST_RES = None


def kernel(Q, length, Wq, bq, Wk, bk, Wv, bv):
    global LAST_EXEC_NS, LAST_RES
    _install_shims()
    from concourse.bass_utils import run_bass_kernel_spmd

    Q = np.asarray(Q, np.float32)
    length = np.asarray(length, np.int32)
    Wq, Wk, Wv = (np.asarray(w, np.float32) for w in (Wq, Wk, Wv))
    bq, bk, bv = (np.asarray(b, np.float32) for b in (bq, bk, bv))

    use_bias = bool(np.any(bq) or np.any(bk) or np.any(bv))

    import ml_dtypes
    bfl = ml_dtypes.bfloat16

    # ---- length-balanced assignment: sort by key-tile count, pair rank g
    # with rank B-1-g; two cores per pair, 3 head-pairs of each batch
    w = np.clip((np.minimum(np.maximum(length, 0), S) + 127) // 128, 1, NST)
    order = np.argsort(-w, kind="stable")
    s_hi = int(w[order[0]])
    s_lo = int(w[order[B // 2]])

    qt_all = np.ascontiguousarray(Q.transpose(0, 2, 1)).astype(bfl)  # [B,768,1024]

    def wtiles(WT, half):
        # [768, 384] slice -> [3, 128, 6, 128] (partition-contiguous rows)
        sl = WT[:, half * OW:(half + 1) * OW]
        return np.ascontiguousarray(
            sl.reshape(NKT, 128, NPAIR, 128).transpose(2, 1, 0, 3)).astype(bfl)

    WqT, WkT, WvT = Wq.T, Wk.T, Wv.T
    j = np.arange(S)
    mb_all = np.where(j[None, :] < length[:, None], 0.0, MASK_BIAS).astype(np.float32)
    mb_all = np.ascontiguousarray(
        mb_all.reshape(B, NST, 128).transpose(0, 2, 1))  # [B, 128, 8]

    nc = _build_program(s_hi, s_lo, use_bias)
    in_maps = []
    core_batches = []
    for g in range(B // 2):
        bh, bl = int(order[g]), int(order[B - 1 - g])
        for half in range(2):
            m = {
                "qt": np.ascontiguousarray(
                    np.stack([qt_all[bh], qt_all[bl]])),
                "wqm": wtiles(WqT, half),
                "wkm": wtiles(WkT, half),
                "wvt": np.ascontiguousarray(
                    WvT[:, half * OW:(half + 1) * OW]
                    .reshape(NKT, 128, OW)).astype(bfl),
                "mb": np.ascontiguousarray(
                    np.stack([mb_all[bh], mb_all[bl]], axis=1)),  # [128,2,8]
            }
            if use_bias:
                for nm, bias in (("bq", bq), ("bk", bk), ("bv", bv)):
                    m[nm] = bias[half * OW:(half + 1) * OW].reshape(1, -1) \
                        .astype(np.float32).astype(bfl)
            in_maps.append(m)
            core_batches.append((bh, bl))

    res = run_bass_kernel_spmd(
        nc, in_maps, core_ids=list(range(NCORES)), trace=TRACE)
    LAST_EXEC_NS = res.exec_time_ns
    LAST_RES = res

    out = np.empty((B, S, D_MODEL), np.float32)
    for c in range(NCORES):
        half = c % 2
        bh, bl = core_batches[c]
        o = res.results[c]["out"]  # [2, 128, NST*OW] partition-major
        for i, bg in enumerate((bh, bl)):
            out[bg, :, half * OW:(half + 1) * OW] = (
                o[i].reshape(128, NST, OW).transpose(1, 0, 2).reshape(S, OW))
    return np.ascontiguousarray(out)


# revision 36
# speedup vs baseline: 1.0917x; 1.0917x over previous
"""Multi-head self-attention kernel for Trainium2, length-balanced over 8 NeuronCores.

Problem: B=8, S=1024, IN_DIM=D_MODEL=768, H=12, DK=64.
  q/k/v = Q @ W{q,k,v}.T + b   -> [b, H, s, dk]
  scores = exp(q k^T / 8) * key_mask ; attn = scores / (sum + 1e-8)
  out = attn @ v -> [b, s, 768]

Sharding: scores/exp/ctx work for batch b scales with w(b)=ceil(length[b]/128)
key tiles. Batches are sorted by w and paired heavy+light (rank g with rank
7-g); each pair of batches is served by two cores, each taking 3 of the 6
head-pair d-tiles of BOTH batches. Every core runs the same SPMD program with
slot bounds [sH,sH,sH,sL,sL,sL] (sH = max heavy w, sL = max light w), so the
per-core key-tile work drops from 6*max(w) to 3*(sH+sL).

Per slot (batch b, local d-tile t):
  - qT/kT [128,1024] via 24 matmuls N=512 (lhsT = weight k-tiles)
  - scoresT[sk,sq] psum via K=64 matmuls (two heads packed in PE rows
    0-63 / 64-127), sk < bound only
  - exp fused with per-partition mask bias + 1/sqrt(dk) scale on ACT -> bf16
  - ctx psum[sq,65] accumulated over sk; col 64 = rowsum (ones col in v);
    DVE normalizes into out_sb
  - v in [s, d] layout, 65-stride heads with ones col, rows only up to bound
  - pipeline: ctx of slot j-1 (and vproj) ride under scores of slot j
Host gathers the per-core [2, 8sq, 128, 384] outputs into [8, 1024, 768].
"""

import functools
import sys
import types

import numpy as np

B, S, IN_DIM, D_MODEL, H = 8, 1024, 768, 768, 12
DK = D_MODEL // H
NCORES = 8
NKT = IN_DIM // 128   # 6 contraction tiles
NDT = D_MODEL // 128  # 6 d-tiles (head pairs)
NST = S // 128        # 8 s-tiles
NPAIR = 3             # head-pairs per core (half of NDT)
VW = NPAIR * 2 * (DK + 1)   # 390: v columns per core-batch (6 heads x 65)
OW = NPAIR * 128            # 384: out columns per core-batch
MASK_BIAS = -60.0


def _install_shims():
    """antenv.axon_hooks shim (for NTFF tracing) + Tile drain-wait splitting
    (this walrus build accepts only one sync-wait command per Drain/CTRL)."""
    if 'antenv.axon_hooks' not in sys.modules:
        mod = types.ModuleType('antenv.axon_hooks')
        mod._hook = None
        mod.set_axon_ntff_profile_hook = lambda h: setattr(mod, '_hook', h)
        mod.get_axon_ntff_profile_hook = lambda: mod._hook
        sys.modules['antenv.axon_hooks'] = mod
        try:
            import antenv
            antenv.axon_hooks = mod
            from trn_agent_boot.trn_boot import _ntff_profile_via_ctypes
            mod.set_axon_ntff_profile_hook(
                _ntff_profile_via_ctypes('/opt/axon/libaxon_pjrt.so'))
        except Exception:
            pass

    import concourse.tile as tile
    if getattr(tile.TileContext, '_drain_patched', False):
        return
    from concourse.vector_clock import ScopedClock, VectorClock

    def _patched_drain_and_barrier(self, tick_clock, wait_clock):
        nc = self.nc
        gvec = tick_clock.global_clock
        n = len(gvec)
        # Only the DMA/collective proc lanes (>=10) complete asynchronously;
        # engine lanes are fenced by the barrier's own arrival. Spread the
        # one-wait-per-NOP chains across queues so they drain in parallel.
        engs = [nc.sync, nc.gpsimd, nc.scalar, nc.vector, nc.tensor]
        ei = 0
        for i in range(n):
            t = gvec[i]
            if t <= 0 or i < 10:
                continue
            v = [0] * n
            v[i] = t
            nop = engs[ei % len(engs)].nop(nofuse=True, hint="drain_wait_split")
            ei += 1
            wait_clock.add_sem_waits(nop.ins, ScopedClock({None: VectorClock(v)}))
        nc.sync.drain()
        nc.all_engine_barrier()
        assert self.sems is not None
        popped = nc._tile_sem_poison_stack.pop()
        assert popped is self._sem_poison
        # Program ends here: skip the semaphore clears + second barrier
        # (pure epilogue hygiene for nested scopes; outputs are already
        # fenced by the drain + barrier above). Keep host-side bookkeeping.
        sems = list(self.sems.allocated().values())
        nc._state.prepend_free_semaphores(
            [s.num if hasattr(s, 'num') else s for s in sems])

    tile.TileContext._drain_and_barrier = _patched_drain_and_barrier

    # This walrus build accepts at most ONE sync-wait command per engine
    # instruction: split extra waits onto non-fusable NOPs emitted just
    # before the instruction on the same engine queue.
    import bass_rust
    import concourse.mybir as mybir
    _orig_lower = tile.TileContext._lower_ordered_insts

    def _split_waits_then_lower(self, ordered):
        nc = self.nc
        for bbname, insts in ordered.items():
            need = any(
                i.sync_info is not None and i.sync_info.on_wait
                and len(i.sync_info.on_wait) > 1
                for i in insts)
            if not need:
                continue
            out = []
            for inst in insts:
                si = inst.sync_info
                if si is not None and si.on_wait and len(si.on_wait) > 1:
                    waits = list(si.on_wait)
                    for w in waits[:-1]:
                        nop = mybir.InstNoOp(
                            name=nc.get_next_instruction_name(), ins=[], outs=[])
                        nop.engine = inst.engine
                        nop.bass_nofuse = True
                        nop.sync_info = bass_rust.SyncInfo(
                            on_wait=[w], on_update=[])
                        out.append(nop)
                    inst.sync_info = bass_rust.SyncInfo(
                        on_wait=[waits[-1]],
                        on_update=list(si.on_update or []))
                out.append(inst)
            insts[:] = out
        return _orig_lower(self, ordered)

    tile.TileContext._lower_ordered_insts = _split_waits_then_lower
    tile.TileContext._drain_patched = True


@functools.lru_cache(maxsize=None)
def _build_program(s_hi: int, s_lo: int, use_bias: bool, add_eps: bool):
    import concourse.bass as bass
    import concourse.tile as tile
    import concourse.mybir as mybir
    from contextlib import ExitStack

    f32 = mybir.dt.float32
    bf16 = mybir.dt.bfloat16
    EXP = mybir.ActivationFunctionType.Exp

    # slot -> (batch index, sk bound); light batch last keeps the tail short
    slots = [(0, s_hi)] * NPAIR + [(1, s_lo)] * NPAIR
    bounds = {0: s_hi, 1: s_lo}

    nc = bass.Bass("TRN2", enable_partition_id=False)
    qt_d = nc.dram_tensor("qt", (2, IN_DIM, S), bf16, kind="ExternalInput")
    wqm_d = nc.dram_tensor("wqm", (NPAIR, 128, NKT, 128), bf16, kind="ExternalInput")
    wkm_d = nc.dram_tensor("wkm", (NPAIR, 128, NKT, 128), bf16, kind="ExternalInput")
    wvt_d = nc.dram_tensor("wvt", (NKT, 128, OW), bf16, kind="ExternalInput")
    mb_d = nc.dram_tensor("mb", (128, 2, NST), f32, kind="ExternalInput")
    if use_bias:
        bq_d = nc.dram_tensor("bq", (1, OW), bf16, kind="ExternalInput")
        bk_d = nc.dram_tensor("bk", (1, OW), bf16, kind="ExternalInput")
        bv_d = nc.dram_tensor("bv", (1, OW), bf16, kind="ExternalInput")
    # partition-major output: [batch][partition, sq*OW + col] so the DMA
    # moves >=3KB contiguous per partition row; host un-permutes
    out_d = nc.dram_tensor("out", (2, 128, NST * OW), f32, kind="ExternalOutput")

    with tile.TileContext(nc) as tc, ExitStack() as ctx:
        const = ctx.enter_context(tc.tile_pool(name="const", bufs=1))
        big = ctx.enter_context(tc.tile_pool(name="big", bufs=1))
        qkpool = ctx.enter_context(tc.tile_pool(name="qk", bufs=3))
        smpool = ctx.enter_context(tc.tile_pool(name="sm", bufs=4))
        pj = ctx.enter_context(tc.tile_pool(name="pj", bufs=4, space="PSUM"))
        sc = ctx.enter_context(tc.tile_pool(name="sc", bufs=2, space="PSUM"))

        # ---- input DMA: critical path first (mb, wq0/wk0, qtA), the rest
        # (wvt, remaining weights) behind; qtB is deferred into slot 0's
        # pipeline so it doesn't steal HBM bandwidth from qtA
        mb_sb = const.tile([128, 2, NST], f32)
        nc.gpsimd.dma_start(out=mb_sb, in_=mb_d[:, :, :])

        wq_big = big.tile([128, NPAIR, NKT, 128], bf16, name="wq")
        wk_big = big.tile([128, NPAIR, NKT, 128], bf16, name="wk")
        wq_sb = [wq_big[:, t] for t in range(NPAIR)]
        wk_sb = [wk_big[:, t] for t in range(NPAIR)]
        nc.scalar.dma_start(out=wq_big[:, 0], in_=wqm_d[0])
        nc.scalar.dma_start(out=wk_big[:, 0], in_=wkm_d[0])

        qt_big = [big.tile([128, NKT, S], bf16, name=f"qt{b}") for b in range(2)]
        qt_sb = [[qt_big[b][:, k] for k in range(NKT)] for b in range(2)]
        for k in range(NKT):
            eng = nc.sync if k % 2 == 0 else nc.gpsimd
            eng.dma_start(out=qt_big[0][:, k], in_=qt_d[0, k * 128:(k + 1) * 128, :])
        wvt_big = big.tile([128, NKT, OW], bf16, name="wvt")
        wvt_sb = [wvt_big[:, k] for k in range(NKT)]
        for k in range(NKT):
            nc.gpsimd.dma_start(out=wvt_big[:, k], in_=wvt_d[k])
        for t in range(1, NPAIR):
            nc.scalar.dma_start(out=wq_big[:, t], in_=wqm_d[t])
            nc.scalar.dma_start(out=wk_big[:, t], in_=wkm_d[t])

        def emit_qtb_dma(k):
            eng = nc.sync if k % 2 == 0 else nc.gpsimd
            eng.dma_start(out=qt_big[1][:, k], in_=qt_d[1, k * 128:(k + 1) * 128, :])

        if use_bias:
            ones_sb = const.tile([1, 512], bf16)
            nc.vector.memset(ones_sb, 1.0)
            bq_sb = const.tile([1, OW], bf16)
            nc.sync.dma_start(out=bq_sb, in_=bq_d[:, :])
            bk_sb = const.tile([1, OW], bf16)
            nc.sync.dma_start(out=bk_sb, in_=bk_d[:, :])
            bv_sb = const.tile([1, OW], bf16)
            nc.sync.dma_start(out=bv_sb, in_=bv_d[:, :])

        v_sb = [big.tile([128, bounds[b], VW], bf16, name=f"vsb{b}")
                for b in range(2)]
        out_sb = [big.tile([128, NST, OW], f32, name=f"out{b}")
                  for b in range(2)]
        probs_big = [big.tile([128, 2, NST, S], bf16, name=f"probs{par}")
                     for par in range(2)]

        # ---- v projection for (batch b, s-row tile): [s, d] layout,
        # heads strided by DK+1 with a ones column appended
        def emit_vproj(b, srow):
            ps = pj.tile([128, NPAIR * 2 * DK], f32, tag="px",
                         name=f"psv{b}_{srow}")
            for k in range(NKT):
                nc.tensor.matmul(
                    ps,
                    lhsT=qt_sb[b][k][:, srow * 128:(srow + 1) * 128],
                    rhs=wvt_sb[k],
                    start=(k == 0), stop=(k == NKT - 1 and not use_bias))
            if use_bias:
                nc.tensor.matmul(
                    ps, lhsT=ones_sb[0:1, 0:128], rhs=bv_sb,
                    start=False, stop=True)
            dst = v_sb[b][:, srow, :]
            dst3 = dst.rearrange("p (h x) -> p h x", x=DK + 1)[:, :, 0:DK]
            src3 = ps.rearrange("p (h x) -> p h x", x=DK)
            nc.vector.tensor_copy(out=dst3, in_=src3)
            ones_dst = dst.rearrange("p (h x) -> p h x", x=DK + 1)[:, :, DK:DK + 1]
            nc.vector.memset(ones_dst, 1.0)

        # ---- qT/kT projection for slot (batch b, local pair t)
        def emit_qkproj(b, t):
            qT = qkpool.tile([128, S], bf16, tag="qT", name=f"qT{b}_{t}")
            kT = qkpool.tile([128, S], bf16, tag="kT", name=f"kT{b}_{t}")
            # kT is only consumed up to bound*128 key columns by scores
            nch_k = -(-(bounds[b] * 128) // 512)
            for w_sb, bias_nm, dstT, nm in (
                    (wq_sb[t], "bq", qT, "q"), (wk_sb[t], "bk", kT, "k")):
                for nch in range(2 if nm == "q" else nch_k):
                    ps = pj.tile([128, 512], f32, tag="px",
                                 name=f"ps{nm}{b}_{t}_{nch}")
                    for k in range(NKT):
                        nc.tensor.matmul(
                            ps,
                            lhsT=w_sb[:, k, :],
                            rhs=qt_sb[b][k][:, nch * 512:(nch + 1) * 512],
                            start=(k == 0), stop=(k == NKT - 1 and not use_bias))
                    if use_bias:
                        bias_sb = bq_sb if bias_nm == "bq" else bk_sb
                        nc.tensor.matmul(
                            ps,
                            lhsT=bias_sb[0:1, t * 128:(t + 1) * 128],
                            rhs=ones_sb[0:1, 0:512],
                            start=False, stop=True)
                    nc.vector.tensor_copy(
                        out=dstT[:, nch * 512:(nch + 1) * 512], in_=ps)
            return qT, kT

        probs = {}

        def emit_scores_sk(j, b, sk, qT, kT):
            pss = []
            for hl in range(2):
                pss.append(sc.tile([128, S], f32, tag="sc",
                                   name=f"sc{j}_{sk}_{hl}"))
            # nch outer / hl inner: adjacent matmuls target different PE
            # row-groups (partitions 0-63 vs 64-127) and run concurrently
            for nch in range(2):
                for hl in range(2):
                    lo, hi = hl * 64, (hl + 1) * 64
                    nc.tensor.matmul(
                        pss[hl][:, nch * 512:(nch + 1) * 512],
                        lhsT=kT[lo:hi, sk * 128:(sk + 1) * 128],
                        rhs=qT[lo:hi, nch * 512:(nch + 1) * 512],
                        start=True, stop=True)
            for hl in range(2):
                pb = probs_big[j % 2][:, hl, sk]
                nc.scalar.activation(
                    out=pb, in_=pss[hl], func=EXP,
                    bias=mb_sb[:, b, sk:sk + 1], scale=1.0 / np.sqrt(DK))
                probs[(j % 2, hl, sk)] = pb

        def emit_ctx_group(j, b, t, g):
            sq, hl = g // 2, g % 2
            hloc = 2 * t + hl
            n_sk = bounds[b]
            pc = pj.tile([128, DK + 1], f32, tag="px", name=f"cx{j}_{g}")
            for sk in range(n_sk):
                nc.tensor.matmul(
                    pc,
                    lhsT=probs[(j % 2, hl, sk)][:, sq * 128:(sq + 1) * 128],
                    rhs=v_sb[b][:, sk, hloc * (DK + 1):(hloc + 1) * (DK + 1)],
                    start=(sk == 0), stop=(sk == n_sk - 1))
            rc = smpool.tile([128, 1], f32, tag="rc", name=f"rc{j}_{g}")
            if add_eps:
                # only needed when a batch can have length 0 (rowsum == 0);
                # otherwise rowsum >= e^-5 and the 1e-8 shifts out < 1e-8 rel
                nc.vector.tensor_scalar_add(rc, pc[:, DK:DK + 1], 1e-8)
                nc.vector.reciprocal(rc, rc)
            else:
                nc.vector.reciprocal(rc, pc[:, DK:DK + 1])
            nc.vector.tensor_scalar_mul(
                out_sb[b][:, sq, hloc * DK:(hloc + 1) * DK], pc[:, 0:DK], rc)

        def emit_out_dma(b, sqh):
            # one DMA per half batch: [128, 4*OW] = 6KB per partition row
            # (bigger descriptors; DMA engines are descriptor-rate bound)
            eng = (nc.gpsimd, nc.sync)[(b * 2 + sqh) % 2]
            eng.dma_start(
                out=out_d[b][:, sqh * 4 * OW:(sqh + 1) * 4 * OW],
                in_=out_sb[b][:, sqh * 4:(sqh + 1) * 4, :])

        # ---- main pipeline: slot j runs qkproj + scores; ctx of slot j-1
        # (and vproj of a freshly started batch) ride along its sk loop
        def carried_work(j):
            """List of closures to interleave under slot j's sk loop."""
            work = []
            if j == 0:
                work += [functools.partial(emit_vproj, 0, sr)
                         for sr in range(s_hi)]
                work += [functools.partial(emit_qtb_dma, k)
                         for k in range(NKT)]
            if j == 2:
                work += [functools.partial(emit_vproj, 1, sr)
                         for sr in range(s_lo)]
            if j > 0:
                pb, pt = slots[j - 1][0], (j - 1) % NPAIR
                for g in range(2 * NST):
                    work.append(functools.partial(emit_ctx_group, j - 1, pb, pt, g))
                    if pt == NPAIR - 1 and g % 8 == 7:
                        # batch pb's out columns complete for this half
                        work.append(functools.partial(emit_out_dma, pb, g // 8))
            return work

        for j, (b, bound) in enumerate(slots):
            qT, kT = emit_qkproj(b, j % NPAIR)
            work = carried_work(j)
            wi = 0
            for sk in range(bound):
                emit_scores_sk(j, b, sk, qT, kT)
                target = len(work) * (sk + 1) // bound
                while wi < target:
                    work[wi]()
                    wi += 1
            while wi < len(work):
                work[wi]()
                wi += 1

        # tail: ctx + output of the last (light) slot
        j = len(slots) - 1
        for g in range(2 * NST):
            emit_ctx_group(j, 1, NPAIR - 1, g)
            if g % 8 == 7:
                emit_out_dma(1, g // 8)

    return nc


TRACE = False
LAST_EXEC_NS = None
LAST_RES = None


def kernel(Q, length, Wq, bq, Wk, bk, Wv, bv):
    global LAST_EXEC_NS, LAST_RES
    _install_shims()
    from concourse.bass_utils import run_bass_kernel_spmd

    Q = np.asarray(Q, np.float32)
    length = np.asarray(length, np.int32)
    Wq, Wk, Wv = (np.asarray(w, np.float32) for w in (Wq, Wk, Wv))
    bq, bk, bv = (np.asarray(b, np.float32) for b in (bq, bk, bv))

    use_bias = bool(np.any(bq) or np.any(bk) or np.any(bv))

    import ml_dtypes
    bfl = ml_dtypes.bfloat16

    # ---- length-balanced assignment: sort by key-tile count, pair rank g
    # with rank B-1-g; two cores per pair, 3 head-pairs of each batch
    w = np.clip((np.minimum(np.maximum(length, 0), S) + 127) // 128, 1, NST)
    order = np.argsort(-w, kind="stable")
    s_hi = int(w[order[0]])
    s_lo = int(w[order[B // 2]])

    qt_all = np.ascontiguousarray(Q.transpose(0, 2, 1)).astype(bfl)  # [B,768,1024]

    def wtiles(WT, half):
        # [768, 384] slice -> [3, 128, 6, 128] (partition-contiguous rows)
        sl = WT[:, half * OW:(half + 1) * OW]
        return np.ascontiguousarray(
            sl.reshape(NKT, 128, NPAIR, 128).transpose(2, 1, 0, 3)).astype(bfl)

    WqT, WkT, WvT = Wq.T, Wk.T, Wv.T
    j = np.arange(S)
    mb_all = np.where(j[None, :] < length[:, None], 0.0, MASK_BIAS).astype(np.float32)
    mb_all = np.ascontiguousarray(
        mb_all.reshape(B, NST, 128).transpose(0, 2, 1))  # [B, 128, 8]

    add_eps = bool((np.asarray(length) < 1).any())
    nc = _build_program(s_hi, s_lo, use_bias, add_eps)
    in_maps = []
    core_batches = []
    for g in range(B // 2):
        bh, bl = int(order[g]), int(order[B - 1 - g])
        for half in range(2):
            m = {
                "qt": np.ascontiguousarray(
                    np.stack([qt_all[bh], qt_all[bl]])),
                "wqm": wtiles(WqT, half),
                "wkm": wtiles(WkT, half),
                "wvt": np.ascontiguousarray(
                    WvT[:, half * OW:(half + 1) * OW]
                    .reshape(NKT, 128, OW)).astype(bfl),
                "mb": np.ascontiguousarray(
                    np.stack([mb_all[bh], mb_all[bl]], axis=1)),  # [128,2,8]
            }
            if use_bias:
                for nm, bias in (("bq", bq), ("bk", bk), ("bv", bv)):
                    m[nm] = bias[half * OW:(half + 1) * OW].reshape(1, -1) \
                        .astype(np.float32).astype(bfl)
            in_maps.append(m)
            core_batches.append((bh, bl))

    res = run_bass_kernel_spmd(
        nc, in_maps, core_ids=list(range(NCORES)), trace=TRACE)
    LAST_EXEC_NS = res.exec_time_ns
    LAST_RES = res

    out = np.empty((B, S, D_MODEL), np.float32)
    for c in range(NCORES):
        half = c % 2
        bh, bl = core_batches[c]
        o = res.results[c]["out"]  # [2, 128, NST*OW] partition-major
        for i, bg in enumerate((bh, bl)):
            out[bg, :, half * OW:(half + 1) * OW] = (
                o[i].reshape(128, NST, OW).transpose(1, 0, 2).reshape(S, OW))
    return np.ascontiguousarray(out)
